# revision 8
# baseline (speedup 1.0000x reference)
"""Two-layer GAT (DGL GATConv) on 8 Trainium2 NeuronCores — v3.

v3 over v2:
  * Epilogues batched: per-group seg PSUM is stashed (one ScalarE copy) into
    a [P, G, W] staging tile; softmax-normalize / elu / log-softmax run as a
    handful of LARGE DVE ops per block of groups instead of ~6 tiny ops per
    group (tiny DVE ops cost 2-8us each in-situ from fixed overhead +
    in-order queue blocking + SBUF-port contention with the Q7 gathers).
  * Layer-2 projection is interleaved into the layer-1 edge loop per block,
    so the layer-2 AllGather (half A) completes before layer-1 finishes.
  * Gathers run in a skewed pipeline: A-half gathers are issued SKEW pairs
    ahead of B-half ones, hiding the B-half AllGather latency.
  * ev/mt elementwise work is pair-granular (half the instruction count).
  * er table stored bf16 (no per-group cast).

One-hot window matrices (oh/ohT) come from the host as bf16; the gathered
row layout, dst-sorted edge schedule, and the skipped softmax
max-subtraction are as in v2.
"""

import math
import os
import sys
from contextlib import ExitStack

import numpy as np

for _p in ("/opt/trn_rl_repo", "/root/.axon_site/_ro/trn_rl_repo"):
    if os.path.isdir(_p) and _p not in sys.path:
        sys.path.append(_p)

import ml_dtypes

import concourse.bass as bass
import concourse.tile as tile
from concourse import bacc, mybir
from concourse.bass_utils import run_bass_kernel_spmd

BF16 = ml_dtypes.bfloat16

N = 50000
E = 800000
F_IN = 128
H, D, C = 4, 32, 47
HD = H * D            # 128
HC = H * C            # 188
NEG_SLOPE = 0.2

NCORES = 8
P = 128
NPC = N // NCORES         # 6250
G = math.ceil(NPC / P)    # 49
NPAD = G * P              # 6272
GA = 25                   # groups in half A
HSPLIT = GA * P           # 3200 rows per core in half A
HB = NPAD - HSPLIT        # 3072 rows per core in half B
NTA = HSPLIT * NCORES     # 25600 (< 2^15)
NTB = HB * NCORES         # 24576 (< 2^15)

W1 = HD + H               # 132
W2 = HC + H               # 192
ELEM = 256                # gathered row width in bf16 -> 512B
SKEW = 4                  # pairs of A-half gathers issued ahead

LAST_EXEC_NS = None


def _schedule(src, dst):
    """Edge schedule: per (core, group, half) slot runs + one-hot tensors."""
    order = np.argsort(dst, kind="stable")
    s_src = src[order].astype(np.int64)
    s_dst = dst[order].astype(np.int64)

    core_of = s_dst // NPC
    g_of = (s_dst % NPC) // P
    win = (s_dst % NPC) % P

    src_c = s_src // NPC               # owner core of src
    src_i = s_src % NPC                # within-core index (< 6250)
    half = (src_i >= HSPLIT).astype(np.int64)
    idx_val = np.where(half == 0, src_c * HSPLIT + src_i,
                       src_c * HB + (src_i - HSPLIT))

    cgh = (core_of * G + g_of) * 2 + half
    order2 = np.argsort(cgh, kind="stable")
    cgh = cgh[order2]
    idx_val = idx_val[order2]
    win = win[order2]

    counts = np.bincount(cgh, minlength=NCORES * G * 2).reshape(NCORES, G, 2)
    KA = int(math.ceil(counts[:, :, 0].max() / P))
    KB = int(math.ceil(counts[:, :, 1].max() / P))
    K = KA + KB

    starts = np.zeros(NCORES * G * 2 + 1, dtype=np.int64)
    np.cumsum(counts.ravel(), out=starts[1:])
    pos_in_run = np.arange(len(cgh)) - starts[cgh]

    base = np.where(cgh % 2 == 0, 0, KA * P)
    flat = base + pos_in_run
    cg = cgh // 2

    idx_flat = np.zeros((NCORES * G, K * P), dtype=np.int64)   # pad -> row 0
    dstl_flat = np.full((NCORES * G, K * P), -1, dtype=np.int64)  # pad -> -1
    idx_flat[cg, flat] = idx_val
    dstl_flat[cg, flat] = win

    idx_flat = idx_flat.reshape(NCORES, G, K, P)
    dstl_flat = dstl_flat.reshape(NCORES, G, K, P)

    def wrap(a):
        # [NC, G, n] slot-major -> [NC, G, 128, n/16] wrapped+replicated
        n = a.shape[-1]
        w = a.reshape(*a.shape[:-1], n // 16, 16)
        w = np.swapaxes(w, -1, -2)                    # [.., 16, n/16]
        return np.tile(w, (1, 1, 8, 1)).astype(np.int16)

    idxA_w = wrap(idx_flat[:, :, :KA, :].reshape(NCORES, G, KA * P))
    idxB_w = wrap(idx_flat[:, :, KA:, :].reshape(NCORES, G, KB * P))

    # one-hots, built per core to bound peak memory
    m_ar = np.arange(P, dtype=np.int64)
    oh = np.empty((NCORES, G, P, K * P), dtype=BF16)
    ohT = np.empty((NCORES, G, P, K * P), dtype=BF16)
    for c in range(NCORES):
        d = dstl_flat[c]                                   # [G, K, P]
        eq = (d[:, :, :, None] == m_ar).astype(BF16)       # [G, K, Pj, Pm]
        oh[c] = eq.transpose(0, 2, 1, 3).reshape(G, P, K * P)   # [j,(k,m)]
        ohT[c] = eq.transpose(0, 3, 1, 2).reshape(G, P, K * P)  # [m,(k,j)]

    return dict(idxA_w=idxA_w, idxB_w=idxB_w, oh=oh, ohT=ohT, KA=KA, KB=KB)


def _blockdiag(a, hd, h, dim):
    out = np.zeros((hd, h), dtype=np.float32)
    for i in range(h):
        out[i * dim:(i + 1) * dim, i] = a[i]
    return out


def _build_program(KA, KB):
    K = KA + KB
    nc = bacc.Bacc("TRN2", target_bir_lowering=False, debug=False,
                   num_devices=NCORES)
    dt = mybir.dt
    f32, bf16, i16 = dt.float32, dt.bfloat16, dt.int16
    AF = mybir.ActivationFunctionType

    def inp(name, shape, d=f32):
        return nc.dram_tensor(name, shape, d, kind="ExternalInput").ap()

    x_own = inp("x_own", [NPAD, F_IN])
    w1cat = inp("w1cat", [F_IN, W1 + H], bf16)
    w2cat = inp("w2cat", [F_IN, W2 + H], bf16)
    b1_t = inp("b1_t", [P, HD])
    b2m_t = inp("b2m_t", [P, C])
    ident_t = inp("ident_t", [P, P])
    idxA_in = inp("idxA_in", [G, P, KA * 8], i16)
    idxB_in = inp("idxB_in", [G, P, KB * 8], i16)
    oh_in = inp("oh_in", [G, P, K * P], bf16)
    ohT_in = inp("ohT_in", [G, P, K * P], bf16)

    y_out = nc.dram_tensor("y_out", [NPAD, C], f32, kind="ExternalOutput").ap()

    tab1_own = nc.dram_tensor("tab1_own", [NPAD, ELEM], bf16).ap()
    tab1A = nc.dram_tensor("tab1A", [NTA, ELEM], bf16, addr_space="Shared").ap()
    tab1B = nc.dram_tensor("tab1B", [NTB, ELEM], bf16, addr_space="Shared").ap()
    er1_d = nc.dram_tensor("er1_d", [NPAD, H], bf16).ap()
    tab2_own = nc.dram_tensor("tab2_own", [NPAD, ELEM], bf16).ap()
    tab2A = nc.dram_tensor("tab2A", [NTA, ELEM], bf16, addr_space="Shared").ap()
    tab2B = nc.dram_tensor("tab2B", [NTB, ELEM], bf16, addr_space="Shared").ap()
    er2_d = nc.dram_tensor("er2_d", [NPAD, H], bf16).ap()

    pairs = [(g, g + 1) if g + 1 < G else (g,) for g in range(0, G, 2)]
    # layer-1 epilogue blocks; a block boundary at 25 lets the layer-2
    # half-A AllGather start once groups 0..24 are projected
    blocks = [(0, 8), (8, 16), (16, 25), (25, 33), (33, 41), (41, 49)]

    with tile.TileContext(nc) as tc, ExitStack() as ctx:
        const = ctx.enter_context(tc.tile_pool(name="const", bufs=1))
        sb = ctx.enter_context(tc.tile_pool(name="sb", bufs=3))
        ohp = ctx.enter_context(tc.tile_pool(name="ohp", bufs=3))
        mtp = ctx.enter_context(tc.tile_pool(name="mtp", bufs=2))
        fin = ctx.enter_context(tc.tile_pool(name="fin", bufs=1))
        gatA = ctx.enter_context(tc.tile_pool(name="gatA", bufs=SKEW + 1))
        gatB = ctx.enter_context(tc.tile_pool(name="gatB", bufs=2))
        ps = ctx.enter_context(tc.tile_pool(name="ps", bufs=2, space="PSUM"))
        psg = ctx.enter_context(tc.tile_pool(name="psg", bufs=2, space="PSUM"))
        big = ctx.enter_context(tc.tile_pool(name="big", bufs=1))

        ident = const.tile([P, P], f32)
        nc.sync.dma_start(ident[:], ident_t[:])
        b1s = const.tile([P, HD], f32)
        nc.sync.dma_start(b1s[:], b1_t[:])
        b2ms = const.tile([P, C], f32)
        nc.sync.dma_start(b2ms[:], b2m_t[:])
        w1 = const.tile([P, W1 + H], bf16)
        nc.sync.dma_start(w1[:], w1cat[:])
        w2 = const.tile([P, W2 + H], bf16)
        nc.sync.dma_start(w2[:], w2cat[:])

        h1 = big.tile([P, G, F_IN], f32)
        stage = big.tile([P, G, W2], bf16)    # seg staging, both layers
        zsb = big.tile([P, G, C], f32)        # layer-2 z - zmax
        ssb = big.tile([P, G], f32)

        # ---------------- projection ----------------
        def project(src_tile_of, wcat, width, tab_own_d, er_d, glo, ghi):
            for g in range(glo, ghi):
                xt = src_tile_of(g)
                xT_ps = ps.tile([F_IN, P], f32, space="PSUM", tag="xT_ps")
                nc.tensor.transpose(xT_ps[:], xt[:], ident[:])
                xT = sb.tile([F_IN, P], bf16, tag="xT")
                nc.scalar.activation(xT[:], xT_ps[:], AF.Copy)
                pr = ps.tile([P, width + H], f32, space="PSUM", tag="proj")
                nc.tensor.matmul(pr[:], lhsT=xT[:], rhs=wcat[:, :width + H],
                                 start=True, stop=True)
                tb = sb.tile([P, width], bf16, tag="tabrow")
                nc.scalar.activation(tb[:], pr[:, :width], AF.Copy)
                nc.sync.dma_start(tab_own_d[g * P:(g + 1) * P, :width], tb[:])
                er = sb.tile([P, H], bf16, tag="errow")
                nc.scalar.activation(er[:], pr[:, width:width + H], AF.Copy)
                nc.sync.dma_start(er_d[g * P:(g + 1) * P, :], er[:])

        def x_tile(g):
            t = sb.tile([P, F_IN], f32, tag="xload")
            nc.sync.dma_start(t[:], x_own[g * P:(g + 1) * P, :])
            return t

        def h1_tile(g):
            return h1[:, g, :]

        def allgather(src_d, dst_d, lo, hi):
            nc.gpsimd.collective_compute(
                "AllGather", mybir.AluOpType.bypass,
                replica_groups=[list(range(NCORES))],
                ins=[src_d[lo:hi, :]], outs=[dst_d[:]])

        # ---------------- edge phase ----------------
        def edge_phase(tabA, tabB, er_d, width, stash_cb):
            pend = {}

            def issue(pi, hkey):
                if pi >= len(pairs):
                    return
                pair = pairs[pi]
                npair = len(pair)
                tab, Kh, idx_in = ((tabA, KA, idxA_in) if hkey == "A"
                                   else (tabB, KB, idxB_in))
                pool = gatA if hkey == "A" else gatB
                it = sb.tile([P, npair * Kh * 8], i16, tag=f"idx{hkey}")
                for gi, g in enumerate(pair):
                    nc.sync.dma_start(
                        it[:, gi * Kh * 8:(gi + 1) * Kh * 8], idx_in[g])
                gt = pool.tile([P, npair * Kh, ELEM], bf16, tag=f"gt{hkey}")
                nc.gpsimd.dma_gather(
                    out_ap=gt[:], in_ap=tab[:],
                    idxs_ap=it[:], num_idxs=npair * Kh * P,
                    num_idxs_reg=npair * Kh * P, elem_size=ELEM,
                    single_packet=False)
                pend[(pi, hkey)] = gt

            for pi in range(SKEW):
                issue(pi, "A")

            for pi, pair in enumerate(pairs):
                issue(pi, "B")
                issue(pi + SKEW, "A")
                gtA_t = pend.pop((pi, "A"))
                gtB_t = pend.pop((pi, "B"))
                npair = len(pair)

                oh_ts, ohT_ts, erw_ts = [], [], []
                for g in pair:
                    oh_t = ohp.tile([P, K, P], bf16, tag="oh")
                    nc.scalar.dma_start(
                        oh_t[:].rearrange("p k m -> p (k m)"), oh_in[g])
                    ohT_t = ohp.tile([P, K, P], bf16, tag="ohT")
                    nc.scalar.dma_start(
                        ohT_t[:].rearrange("p k m -> p (k m)"), ohT_in[g])
                    erw = sb.tile([P, H], bf16, tag="erw")
                    nc.sync.dma_start(erw[:], er_d[g * P:(g + 1) * P, :])
                    oh_ts.append(oh_t)
                    ohT_ts.append(ohT_t)
                    erw_ts.append(erw)

                mts = {}
                for hkey, Kh, coh, gt in (("A", KA, 0, gtA_t),
                                          ("B", KB, KA, gtB_t)):
                    nk = npair * Kh
                    erp = psg.tile([P, nk, H], f32, space="PSUM", tag="erp")
                    for gi in range(npair):
                        for c in range(Kh):
                            nc.tensor.matmul(
                                erp[:, gi * Kh + c, :],
                                lhsT=ohT_ts[gi][:, coh + c, :],
                                rhs=erw_ts[gi][:], start=True, stop=True)
                    ev = sb.tile([P, nk, H], f32, tag=f"ev{hkey}")
                    nc.vector.tensor_tensor(
                        out=ev[:], in0=gt[:, :, width - H:width],
                        in1=erp[:], op=mybir.AluOpType.add)
                    nc.vector.scalar_tensor_tensor(
                        out=ev[:], in0=ev[:], scalar=NEG_SLOPE, in1=ev[:],
                        op0=mybir.AluOpType.mult, op1=mybir.AluOpType.max)
                    mt = mtp.tile([P, nk, width], bf16, tag=f"mt{hkey}")
                    nc.scalar.activation(
                        mt[:, :, width - H:width], ev[:], AF.Exp)
                    nc.vector.tensor_tensor(
                        out=mt[:, :, :width - H].rearrange(
                            "p k (h d) -> p k h d", h=H),
                        in0=gt[:, :, :width - H].rearrange(
                            "p k (h d) -> p k h d", h=H),
                        in1=mt[:, :, width - H:width, None].to_broadcast(
                            [P, nk, H, (width - H) // H]),
                        op=mybir.AluOpType.mult)
                    mts[hkey] = mt

                for gi, g in enumerate(pair):
                    seg = psg.tile([P, width], f32, space="PSUM", tag="seg")
                    for bi, (hkey, Kh, coh) in enumerate(
                            (("A", KA, 0), ("B", KB, KA))):
                        mt = mts[hkey]
                        for c in range(Kh):
                            nc.tensor.matmul(
                                seg[:], lhsT=oh_ts[gi][:, coh + c, :],
                                rhs=mt[:, gi * Kh + c, :],
                                start=(bi == 0 and c == 0),
                                stop=(bi == 1 and c == Kh - 1))
                    stash_cb(g, seg)

        # ---------------- layer epilogues (batched) ----------------
        def stash1(g, seg):
            nc.scalar.activation(stage[:, g, :W1], seg[:, :W1], AF.Copy)
            for (g0, g1) in blocks:
                if g == g1 - 1:
                    l1_finale(g0, g1)

        def l1_finale(g0, g1):
            nb = g1 - g0
            V = stage[:, g0:g1, :]
            dn = sb.tile([P, nb, H], f32, tag="dn")
            nc.vector.tensor_scalar_max(dn[:], V[:, :, HD:HD + H], 1e-30)
            rd = sb.tile([P, nb, H], f32, tag="rd")
            nc.vector.reciprocal(rd[:], dn[:])
            ht = fin.tile([P, nb, F_IN], f32, tag="ht")
            nc.vector.tensor_tensor(
                out=ht[:].rearrange("p g (h d) -> p g h d", h=H),
                in0=V[:, :, :HD].rearrange("p g (h d) -> p g h d", h=H),
                in1=rd[:, :, :, None].to_broadcast([P, nb, H, D]),
                op=mybir.AluOpType.mult)
            nc.vector.tensor_tensor(
                out=ht[:], in0=ht[:],
                in1=b1s[:, None, :].to_broadcast([P, nb, HD]),
                op=mybir.AluOpType.add)
            mn = fin.tile([P, nb, F_IN], f32, tag="mn")
            nc.vector.tensor_scalar_min(mn[:], ht[:], 0.0)
            nc.scalar.activation(mn[:], mn[:], AF.Exp)
            nc.vector.scalar_tensor_tensor(
                out=h1[:, g0:g1, :], in0=mn[:], scalar=-1.0, in1=ht[:],
                op0=mybir.AluOpType.add, op1=mybir.AluOpType.max)
            # layer-2 projection for the completed block
            project(h1_tile, w2, W2, tab2_own, er2_d, g0, g1)
            if g1 == GA:
                allgather(tab2_own, tab2A, 0, HSPLIT)
            if g1 == G:
                allgather(tab2_own, tab2B, HSPLIT, NPAD)

        def stash2(g, seg):
            nc.scalar.activation(stage[:, g, :W2], seg[:, :W2], AF.Copy)
            if g == GA - 1:
                l2_finale(0, GA)
            elif g == G - 1:
                l2_finale(GA, G)

        def l2_finale(g0, g1):
            nb = g1 - g0
            V = stage[:, g0:g1, :]
            dn = fin.tile([P, nb, H], f32, tag="dn2")
            nc.vector.tensor_scalar_max(dn[:], V[:, :, HC:HC + H], 1e-30)
            rd = fin.tile([P, nb, H], f32, tag="rd2")
            nc.vector.reciprocal(rd[:], dn[:])
            nc.vector.tensor_scalar_mul(rd[:], rd[:], 1.0 / H)
            nc.vector.tensor_tensor(
                out=V[:, :, :HC].rearrange("p g (h c) -> p g h c", h=H),
                in0=V[:, :, :HC].rearrange("p g (h c) -> p g h c", h=H),
                in1=rd[:, :, :, None].to_broadcast([P, nb, H, C]),
                op=mybir.AluOpType.mult)
            zv = zsb[:, g0:g1, :]
            nc.vector.reduce_sum(
                zv, V[:, :, :HC].rearrange("p g (h c) -> p g c h", h=H),
                axis=mybir.AxisListType.X)
            nc.vector.tensor_tensor(
                out=zv, in0=zv,
                in1=b2ms[:, None, :].to_broadcast([P, nb, C]),
                op=mybir.AluOpType.add)
            zm = fin.tile([P, nb], f32, tag="zm")
            nc.vector.reduce_max(zm[:], zv, axis=mybir.AxisListType.X)
            nc.vector.tensor_tensor(
                out=zv, in0=zv,
                in1=zm[:, :, None].to_broadcast([P, nb, C]),
                op=mybir.AluOpType.subtract)
            for g in range(g0, g1):
                es = sb.tile([P, C], f32, tag="es")
                nc.scalar.activation(es[:], zsb[:, g, :], AF.Exp,
                                     accum_out=ssb[:, g:g + 1])
            lg = fin.tile([P, nb], f32, tag="lg")
            nc.scalar.activation(lg[:], ssb[:, g0:g1], AF.Ln)
            yt = fin.tile([P, nb, C], f32, tag="yt")
            nc.vector.tensor_tensor(
                out=yt[:], in0=zv,
                in1=lg[:, :, None].to_broadcast([P, nb, C]),
                op=mybir.AluOpType.subtract)
            nc.sync.dma_start(
                y_out[g0 * P:g1 * P].rearrange("(g p) c -> p g c", p=P),
                yt[:])

        # ---------------- run the two layers ----------------
        project(x_tile, w1, W1, tab1_own, er1_d, 0, GA)
        allgather(tab1_own, tab1A, 0, HSPLIT)
        project(x_tile, w1, W1, tab1_own, er1_d, GA, G)
        allgather(tab1_own, tab1B, HSPLIT, NPAD)
        edge_phase(tab1A, tab1B, er1_d, W1, stash1)
        edge_phase(tab2A, tab2B, er2_d, W2, stash2)

    nc.compile()
    return nc


def kernel(x, src, dst, W1s, W1d, al1, ar1, b1, W2s, W2d, al2, ar2, b2):
    global LAST_EXEC_NS
    x = np.asarray(x, dtype=np.float32)
    src = np.asarray(src, dtype=np.int32)
    dst = np.asarray(dst, dtype=np.int32)

    sch = _schedule(src, dst)
    KA, KB = sch["KA"], sch["KB"]

    def wcat(Ws, Wd, al, ar, hd, h, dim):
        Ws = np.asarray(Ws, np.float32)
        Wd = np.asarray(Wd, np.float32)
        wel = Ws @ _blockdiag(np.asarray(al, np.float32), hd, h, dim)
        wer = Wd @ _blockdiag(np.asarray(ar, np.float32), hd, h, dim)
        return np.concatenate([Ws, wel, wer], axis=1)

    w1c = wcat(W1s, W1d, al1, ar1, HD, H, D).astype(BF16)
    w2c = wcat(W2s, W2d, al2, ar2, HC, H, C).astype(BF16)

    ident_np = np.eye(P, dtype=np.float32)
    b1_np = np.tile(np.asarray(b1, np.float32)[None, :], (P, 1))
    b2m_np = np.tile(np.asarray(b2, np.float32).reshape(H, C).mean(0)[None, :],
                     (P, 1))

    x_pad = np.zeros((NCORES, NPAD, F_IN), np.float32)
    x_pad[:, :NPC, :] = x.reshape(NCORES, NPC, F_IN)

    nc = _build_program(KA, KB)

    in_maps = []
    for c in range(NCORES):
        in_maps.append({
            "x_own": x_pad[c],
            "w1cat": w1c, "w2cat": w2c,
            "b1_t": b1_np, "b2m_t": b2m_np,
            "ident_t": ident_np,
            "idxA_in": sch["idxA_w"][c], "idxB_in": sch["idxB_w"][c],
            "oh_in": sch["oh"][c], "ohT_in": sch["ohT"][c],
        })

    res = run_bass_kernel_spmd(nc, in_maps, list(range(NCORES)),
                               trace=bool(os.environ.get("GAT_TRACE")))
    LAST_EXEC_NS = res.exec_time_ns
    out = np.concatenate(
        [res.results[c]["y_out"][:NPC] for c in range(NCORES)], axis=0)
    return out.astype(np.float32)


# revision 9
# speedup vs baseline: 1.0399x; 1.0399x over previous
"""Two-layer GAT (DGL GATConv) on 8 Trainium2 NeuronCores — v3.

v3 over v2:
  * Epilogues batched: per-group seg PSUM is stashed (one ScalarE copy) into
    a [P, G, W] staging tile; softmax-normalize / elu / log-softmax run as a
    handful of LARGE DVE ops per block of groups instead of ~6 tiny ops per
    group (tiny DVE ops cost 2-8us each in-situ from fixed overhead +
    in-order queue blocking + SBUF-port contention with the Q7 gathers).
  * Layer-2 projection is interleaved into the layer-1 edge loop per block,
    so the layer-2 AllGather (half A) completes before layer-1 finishes.
  * Gathers run in a skewed pipeline: A-half gathers are issued SKEW pairs
    ahead of B-half ones, hiding the B-half AllGather latency.
  * ev/mt elementwise work is pair-granular (half the instruction count).
  * er table stored bf16 (no per-group cast).

One-hot window matrices (oh/ohT) come from the host as bf16; the gathered
row layout, dst-sorted edge schedule, and the skipped softmax
max-subtraction are as in v2.
"""

import math
import os
import sys
from contextlib import ExitStack

import numpy as np

for _p in ("/opt/trn_rl_repo", "/root/.axon_site/_ro/trn_rl_repo"):
    if os.path.isdir(_p) and _p not in sys.path:
        sys.path.append(_p)

import ml_dtypes

import concourse.bass as bass
import concourse.tile as tile
from concourse import bacc, mybir
from concourse.bass_utils import run_bass_kernel_spmd

BF16 = ml_dtypes.bfloat16

N = 50000
E = 800000
F_IN = 128
H, D, C = 4, 32, 47
HD = H * D            # 128
HC = H * C            # 188
NEG_SLOPE = 0.2

NCORES = 8
P = 128
NPC = N // NCORES         # 6250
G = math.ceil(NPC / P)    # 49
NPAD = G * P              # 6272
GA = 25                   # groups in half A
HSPLIT = GA * P           # 3200 rows per core in half A
HB = NPAD - HSPLIT        # 3072 rows per core in half B
NTA = HSPLIT * NCORES     # 25600 (< 2^15)
NTB = HB * NCORES         # 24576 (< 2^15)

W1 = HD + H               # 132
W2 = HC + H               # 192
ELEM = 256                # gathered row width in bf16 -> 512B
SKEW = 5                  # pairs of A-half gathers issued ahead

LAST_EXEC_NS = None


def _schedule(src, dst):
    """Edge schedule: per (core, group, half) slot runs + one-hot tensors."""
    order = np.argsort(dst, kind="stable")
    s_src = src[order].astype(np.int64)
    s_dst = dst[order].astype(np.int64)

    core_of = s_dst // NPC
    g_of = (s_dst % NPC) // P
    win = (s_dst % NPC) % P

    src_c = s_src // NPC               # owner core of src
    src_i = s_src % NPC                # within-core index (< 6250)
    half = (src_i >= HSPLIT).astype(np.int64)
    idx_val = np.where(half == 0, src_c * HSPLIT + src_i,
                       src_c * HB + (src_i - HSPLIT))

    cgh = (core_of * G + g_of) * 2 + half
    order2 = np.argsort(cgh, kind="stable")
    cgh = cgh[order2]
    idx_val = idx_val[order2]
    win = win[order2]

    counts = np.bincount(cgh, minlength=NCORES * G * 2).reshape(NCORES, G, 2)
    KA = int(math.ceil(counts[:, :, 0].max() / P))
    KB = int(math.ceil(counts[:, :, 1].max() / P))
    K = KA + KB

    starts = np.zeros(NCORES * G * 2 + 1, dtype=np.int64)
    np.cumsum(counts.ravel(), out=starts[1:])
    pos_in_run = np.arange(len(cgh)) - starts[cgh]

    base = np.where(cgh % 2 == 0, 0, KA * P)
    flat = base + pos_in_run
    cg = cgh // 2

    idx_flat = np.zeros((NCORES * G, K * P), dtype=np.int64)   # pad -> row 0
    dstl_flat = np.full((NCORES * G, K * P), -1, dtype=np.int64)  # pad -> -1
    idx_flat[cg, flat] = idx_val
    dstl_flat[cg, flat] = win

    idx_flat = idx_flat.reshape(NCORES, G, K, P)
    dstl_flat = dstl_flat.reshape(NCORES, G, K, P)

    def wrap(a):
        # [NC, G, n] slot-major -> [NC, G, 128, n/16] wrapped+replicated
        n = a.shape[-1]
        w = a.reshape(*a.shape[:-1], n // 16, 16)
        w = np.swapaxes(w, -1, -2)                    # [.., 16, n/16]
        return np.tile(w, (1, 1, 8, 1)).astype(np.int16)

    idxA_w = wrap(idx_flat[:, :, :KA, :].reshape(NCORES, G, KA * P))
    idxB_w = wrap(idx_flat[:, :, KA:, :].reshape(NCORES, G, KB * P))

    # one-hots, built per core to bound peak memory
    m_ar = np.arange(P, dtype=np.int64)
    oh = np.empty((NCORES, G, P, K * P), dtype=BF16)
    ohT = np.empty((NCORES, G, P, K * P), dtype=BF16)
    for c in range(NCORES):
        d = dstl_flat[c]                                   # [G, K, P]
        eq = (d[:, :, :, None] == m_ar).astype(BF16)       # [G, K, Pj, Pm]
        oh[c] = eq.transpose(0, 2, 1, 3).reshape(G, P, K * P)   # [j,(k,m)]
        ohT[c] = eq.transpose(0, 3, 1, 2).reshape(G, P, K * P)  # [m,(k,j)]

    return dict(idxA_w=idxA_w, idxB_w=idxB_w, oh=oh, ohT=ohT, KA=KA, KB=KB)


def _blockdiag(a, hd, h, dim):
    out = np.zeros((hd, h), dtype=np.float32)
    for i in range(h):
        out[i * dim:(i + 1) * dim, i] = a[i]
    return out


def _build_program(KA, KB):
    K = KA + KB
    nc = bacc.Bacc("TRN2", target_bir_lowering=False, debug=False,
                   num_devices=NCORES)
    dt = mybir.dt
    f32, bf16, i16 = dt.float32, dt.bfloat16, dt.int16
    AF = mybir.ActivationFunctionType

    def inp(name, shape, d=f32):
        return nc.dram_tensor(name, shape, d, kind="ExternalInput").ap()

    x_own = inp("x_own", [NPAD, F_IN], bf16)
    w1cat = inp("w1cat", [F_IN, W1 + H], bf16)
    w2cat = inp("w2cat", [F_IN, W2 + H], bf16)
    b1_t = inp("b1_t", [P, HD])
    b2m_t = inp("b2m_t", [P, C])
    ident_t = inp("ident_t", [P, P])
    idxA_in = inp("idxA_in", [G, P, KA * 8], i16)
    idxB_in = inp("idxB_in", [G, P, KB * 8], i16)
    oh_in = inp("oh_in", [G, P, K * P], bf16)
    ohT_in = inp("ohT_in", [G, P, K * P], bf16)

    y_out = nc.dram_tensor("y_out", [NPAD, C], f32, kind="ExternalOutput").ap()

    tab1_own = nc.dram_tensor("tab1_own", [NPAD, ELEM], bf16).ap()
    tab1A = nc.dram_tensor("tab1A", [NTA, ELEM], bf16, addr_space="Shared").ap()
    tab1B = nc.dram_tensor("tab1B", [NTB, ELEM], bf16, addr_space="Shared").ap()
    er1_d = nc.dram_tensor("er1_d", [NPAD, H], bf16).ap()
    tab2_own = nc.dram_tensor("tab2_own", [NPAD, ELEM], bf16).ap()
    tab2A = nc.dram_tensor("tab2A", [NTA, ELEM], bf16, addr_space="Shared").ap()
    tab2B = nc.dram_tensor("tab2B", [NTB, ELEM], bf16, addr_space="Shared").ap()
    er2_d = nc.dram_tensor("er2_d", [NPAD, H], bf16).ap()

    pairs = [(g, g + 1) if g + 1 < G else (g,) for g in range(0, G, 2)]
    # layer-1 epilogue blocks; a block boundary at 25 lets the layer-2
    # half-A AllGather start once groups 0..24 are projected
    blocks = [(0, 8), (8, 16), (16, 25), (25, 33), (33, 41), (41, 49)]

    with tile.TileContext(nc) as tc, ExitStack() as ctx:
        const = ctx.enter_context(tc.tile_pool(name="const", bufs=1))
        sb = ctx.enter_context(tc.tile_pool(name="sb", bufs=3))
        ohp = ctx.enter_context(tc.tile_pool(name="ohp", bufs=3))
        mtp = ctx.enter_context(tc.tile_pool(name="mtp", bufs=2))
        fin = ctx.enter_context(tc.tile_pool(name="fin", bufs=1))
        gatA = ctx.enter_context(tc.tile_pool(name="gatA", bufs=SKEW + 1))
        gatB = ctx.enter_context(tc.tile_pool(name="gatB", bufs=2))
        ps = ctx.enter_context(tc.tile_pool(name="ps", bufs=2, space="PSUM"))
        psg = ctx.enter_context(tc.tile_pool(name="psg", bufs=2, space="PSUM"))
        big = ctx.enter_context(tc.tile_pool(name="big", bufs=1))

        ident = const.tile([P, P], f32)
        nc.sync.dma_start(ident[:], ident_t[:])
        b1s = const.tile([P, HD], f32)
        nc.sync.dma_start(b1s[:], b1_t[:])
        b2ms = const.tile([P, C], f32)
        nc.sync.dma_start(b2ms[:], b2m_t[:])
        w1 = const.tile([P, W1 + H], bf16)
        nc.sync.dma_start(w1[:], w1cat[:])
        w2 = const.tile([P, W2 + H], bf16)
        nc.sync.dma_start(w2[:], w2cat[:])

        h1 = big.tile([P, G, F_IN], f32)
        stage = big.tile([P, G, W2], bf16)    # seg staging, both layers
        zsb = big.tile([P, G, C], f32)        # layer-2 z - zmax
        ssb = big.tile([P, G], f32)

        # ---------------- projection ----------------
        def project(lhsT_of, wcat, width, tab_own_d, er_d, glo, ghi):
            for g in range(glo, ghi):
                xT = lhsT_of(g)
                pr = ps.tile([P, width + H], f32, space="PSUM", tag="proj")
                nc.tensor.matmul(pr[:], lhsT=xT[:], rhs=wcat[:, :width + H],
                                 start=True, stop=True)
                tb = sb.tile([P, width], bf16, tag="tabrow")
                nc.scalar.activation(tb[:], pr[:, :width], AF.Copy)
                nc.sync.dma_start(tab_own_d[g * P:(g + 1) * P, :width], tb[:])
                er = sb.tile([P, H], bf16, tag="errow")
                nc.scalar.activation(er[:], pr[:, width:width + H], AF.Copy)
                nc.sync.dma_start(er_d[g * P:(g + 1) * P, :], er[:])

        xT_half = {}

        def load_xT_half(hkey, lo, hi):
            t = fin.tile([P, HSPLIT], bf16, tag="xTall")
            nc.sync.dma_start_transpose(t[:, :hi - lo], x_own[lo:hi, :])
            xT_half[hkey] = (t, lo)

        def x_lhsT(g):
            t, lo = xT_half["A" if g < GA else "B"]
            return t[:, g * P - lo:(g + 1) * P - lo]

        def h1_lhsT(g):
            xt = h1[:, g, :]
            xT_ps = ps.tile([F_IN, P], f32, space="PSUM", tag="xT_ps")
            nc.tensor.transpose(xT_ps[:], xt[:], ident[:])
            xT = sb.tile([F_IN, P], bf16, tag="xT")
            nc.scalar.activation(xT[:], xT_ps[:], AF.Copy)
            return xT[:]

        def allgather(src_d, dst_d, lo, hi):
            nc.gpsimd.collective_compute(
                "AllGather", mybir.AluOpType.bypass,
                replica_groups=[list(range(NCORES))],
                ins=[src_d[lo:hi, :]], outs=[dst_d[:]])

        # ---------------- edge phase ----------------
        def edge_phase(tabA, tabB, er_d, width, stash_cb):
            pend = {}

            def issue(pi, hkey):
                if pi >= len(pairs):
                    return
                pair = pairs[pi]
                npair = len(pair)
                tab, Kh, idx_in = ((tabA, KA, idxA_in) if hkey == "A"
                                   else (tabB, KB, idxB_in))
                pool = gatA if hkey == "A" else gatB
                it = sb.tile([P, npair * Kh * 8], i16, tag=f"idx{hkey}")
                for gi, g in enumerate(pair):
                    nc.sync.dma_start(
                        it[:, gi * Kh * 8:(gi + 1) * Kh * 8], idx_in[g])
                gt = pool.tile([P, npair * Kh, ELEM], bf16, tag=f"gt{hkey}")
                nc.gpsimd.dma_gather(
                    out_ap=gt[:], in_ap=tab[:],
                    idxs_ap=it[:], num_idxs=npair * Kh * P,
                    num_idxs_reg=npair * Kh * P, elem_size=ELEM,
                    single_packet=False)
                pend[(pi, hkey)] = gt

            for pi in range(SKEW):
                issue(pi, "A")

            for pi, pair in enumerate(pairs):
                issue(pi, "B")
                issue(pi + SKEW, "A")
                gtA_t = pend.pop((pi, "A"))
                gtB_t = pend.pop((pi, "B"))
                npair = len(pair)

                oh_ts, ohT_ts, erw_ts = [], [], []
                for g in pair:
                    oh_t = ohp.tile([P, K, P], bf16, tag="oh")
                    nc.sync.dma_start(
                        oh_t[:].rearrange("p k m -> p (k m)"), oh_in[g])
                    ohT_t = ohp.tile([P, K, P], bf16, tag="ohT")
                    nc.sync.dma_start(
                        ohT_t[:].rearrange("p k m -> p (k m)"), ohT_in[g])
                    erw = sb.tile([P, H], bf16, tag="erw")
                    nc.sync.dma_start(erw[:], er_d[g * P:(g + 1) * P, :])
                    oh_ts.append(oh_t)
                    ohT_ts.append(ohT_t)
                    erw_ts.append(erw)

                mts = {}
                for hkey, Kh, coh, gt in (("A", KA, 0, gtA_t),
                                          ("B", KB, KA, gtB_t)):
                    nk = npair * Kh
                    erp = psg.tile([P, nk, H], f32, space="PSUM", tag="erp")
                    for gi in range(npair):
                        for c in range(Kh):
                            nc.tensor.matmul(
                                erp[:, gi * Kh + c, :],
                                lhsT=ohT_ts[gi][:, coh + c, :],
                                rhs=erw_ts[gi][:], start=True, stop=True)
                    ev = sb.tile([P, nk, H], f32, tag=f"ev{hkey}")
                    nc.vector.tensor_tensor(
                        out=ev[:], in0=gt[:, :, width - H:width],
                        in1=erp[:], op=mybir.AluOpType.add)
                    nc.vector.scalar_tensor_tensor(
                        out=ev[:], in0=ev[:], scalar=NEG_SLOPE, in1=ev[:],
                        op0=mybir.AluOpType.mult, op1=mybir.AluOpType.max)
                    mt = mtp.tile([P, nk, width], bf16, tag=f"mt{hkey}")
                    nc.scalar.activation(
                        mt[:, :, width - H:width], ev[:], AF.Exp)
                    nc.vector.tensor_tensor(
                        out=mt[:, :, :width - H].rearrange(
                            "p k (h d) -> p k h d", h=H),
                        in0=gt[:, :, :width - H].rearrange(
                            "p k (h d) -> p k h d", h=H),
                        in1=mt[:, :, width - H:width, None].to_broadcast(
                            [P, nk, H, (width - H) // H]),
                        op=mybir.AluOpType.mult)
                    mts[hkey] = mt

                for gi, g in enumerate(pair):
                    seg = psg.tile([P, width], f32, space="PSUM", tag="seg")
                    for bi, (hkey, Kh, coh) in enumerate(
                            (("A", KA, 0), ("B", KB, KA))):
                        mt = mts[hkey]
                        for c in range(Kh):
                            nc.tensor.matmul(
                                seg[:], lhsT=oh_ts[gi][:, coh + c, :],
                                rhs=mt[:, gi * Kh + c, :],
                                start=(bi == 0 and c == 0),
                                stop=(bi == 1 and c == Kh - 1))
                    stash_cb(g, seg)

        # ---------------- layer epilogues (batched) ----------------
        def stash1(g, seg):
            nc.scalar.activation(stage[:, g, :W1], seg[:, :W1], AF.Copy)
            for (g0, g1) in blocks:
                if g == g1 - 1:
                    l1_finale(g0, g1)

        def l1_finale(g0, g1):
            nb = g1 - g0
            V = stage[:, g0:g1, :]
            dn = sb.tile([P, nb, H], f32, tag="dn")
            nc.vector.tensor_scalar_max(dn[:], V[:, :, HD:HD + H], 1e-30)
            rd = sb.tile([P, nb, H], f32, tag="rd")
            nc.vector.reciprocal(rd[:], dn[:])
            ht = fin.tile([P, nb, F_IN], f32, tag="ht")
            nc.vector.tensor_tensor(
                out=ht[:].rearrange("p g (h d) -> p g h d", h=H),
                in0=V[:, :, :HD].rearrange("p g (h d) -> p g h d", h=H),
                in1=rd[:, :, :, None].to_broadcast([P, nb, H, D]),
                op=mybir.AluOpType.mult)
            nc.vector.tensor_tensor(
                out=ht[:], in0=ht[:],
                in1=b1s[:, None, :].to_broadcast([P, nb, HD]),
                op=mybir.AluOpType.add)
            mn = fin.tile([P, nb, F_IN], f32, tag="mn")
            nc.vector.tensor_scalar_min(mn[:], ht[:], 0.0)
            nc.scalar.activation(mn[:], mn[:], AF.Exp)
            nc.vector.scalar_tensor_tensor(
                out=h1[:, g0:g1, :], in0=mn[:], scalar=-1.0, in1=ht[:],
                op0=mybir.AluOpType.add, op1=mybir.AluOpType.max)
            # layer-2 projection for the completed block
            project(h1_lhsT, w2, W2, tab2_own, er2_d, g0, g1)
            if g1 == GA:
                allgather(tab2_own, tab2A, 0, HSPLIT)
            if g1 == G:
                allgather(tab2_own, tab2B, HSPLIT, NPAD)

        def stash2(g, seg):
            nc.scalar.activation(stage[:, g, :W2], seg[:, :W2], AF.Copy)
            if g == GA - 1:
                l2_finale(0, GA)
            elif g == G - 1:
                l2_finale(GA, G)

        def l2_finale(g0, g1):
            nb = g1 - g0
            V = stage[:, g0:g1, :]
            dn = fin.tile([P, nb, H], f32, tag="dn2")
            nc.vector.tensor_scalar_max(dn[:], V[:, :, HC:HC + H], 1e-30)
            rd = fin.tile([P, nb, H], f32, tag="rd2")
            nc.vector.reciprocal(rd[:], dn[:])
            nc.vector.tensor_scalar_mul(rd[:], rd[:], 1.0 / H)
            nc.vector.tensor_tensor(
                out=V[:, :, :HC].rearrange("p g (h c) -> p g h c", h=H),
                in0=V[:, :, :HC].rearrange("p g (h c) -> p g h c", h=H),
                in1=rd[:, :, :, None].to_broadcast([P, nb, H, C]),
                op=mybir.AluOpType.mult)
            zv = zsb[:, g0:g1, :]
            nc.vector.reduce_sum(
                zv, V[:, :, :HC].rearrange("p g (h c) -> p g c h", h=H),
                axis=mybir.AxisListType.X)
            nc.vector.tensor_tensor(
                out=zv, in0=zv,
                in1=b2ms[:, None, :].to_broadcast([P, nb, C]),
                op=mybir.AluOpType.add)
            zm = fin.tile([P, nb], f32, tag="zm")
            nc.vector.reduce_max(zm[:], zv, axis=mybir.AxisListType.X)
            nc.vector.tensor_tensor(
                out=zv, in0=zv,
                in1=zm[:, :, None].to_broadcast([P, nb, C]),
                op=mybir.AluOpType.subtract)
            for g in range(g0, g1):
                es = sb.tile([P, C], f32, tag="es")
                nc.scalar.activation(es[:], zsb[:, g, :], AF.Exp,
                                     accum_out=ssb[:, g:g + 1])
            lg = fin.tile([P, nb], f32, tag="lg")
            nc.scalar.activation(lg[:], ssb[:, g0:g1], AF.Ln)
            yt = fin.tile([P, nb, C], f32, tag="yt")
            nc.vector.tensor_tensor(
                out=yt[:], in0=zv,
                in1=lg[:, :, None].to_broadcast([P, nb, C]),
                op=mybir.AluOpType.subtract)
            nc.sync.dma_start(
                y_out[g0 * P:g1 * P].rearrange("(g p) c -> p g c", p=P),
                yt[:])

        # ---------------- run the two layers ----------------
        load_xT_half("A", 0, HSPLIT)
        project(x_lhsT, w1, W1, tab1_own, er1_d, 0, GA)
        allgather(tab1_own, tab1A, 0, HSPLIT)
        load_xT_half("B", HSPLIT, NPAD)
        project(x_lhsT, w1, W1, tab1_own, er1_d, GA, G)
        allgather(tab1_own, tab1B, HSPLIT, NPAD)
        edge_phase(tab1A, tab1B, er1_d, W1, stash1)
        edge_phase(tab2A, tab2B, er2_d, W2, stash2)

    nc.compile()
    return nc


def kernel(x, src, dst, W1s, W1d, al1, ar1, b1, W2s, W2d, al2, ar2, b2):
    global LAST_EXEC_NS
    x = np.asarray(x, dtype=np.float32)
    src = np.asarray(src, dtype=np.int32)
    dst = np.asarray(dst, dtype=np.int32)

    sch = _schedule(src, dst)
    KA, KB = sch["KA"], sch["KB"]

    def wcat(Ws, Wd, al, ar, hd, h, dim):
        Ws = np.asarray(Ws, np.float32)
        Wd = np.asarray(Wd, np.float32)
        wel = Ws @ _blockdiag(np.asarray(al, np.float32), hd, h, dim)
        wer = Wd @ _blockdiag(np.asarray(ar, np.float32), hd, h, dim)
        return np.concatenate([Ws, wel, wer], axis=1)

    w1c = wcat(W1s, W1d, al1, ar1, HD, H, D).astype(BF16)
    w2c = wcat(W2s, W2d, al2, ar2, HC, H, C).astype(BF16)

    ident_np = np.eye(P, dtype=np.float32)
    b1_np = np.tile(np.asarray(b1, np.float32)[None, :], (P, 1))
    b2m_np = np.tile(np.asarray(b2, np.float32).reshape(H, C).mean(0)[None, :],
                     (P, 1))

    x_pad = np.zeros((NCORES, NPAD, F_IN), np.float32)
    x_pad[:, :NPC, :] = x.reshape(NCORES, NPC, F_IN)
    x_pad = x_pad.astype(BF16)

    nc = _build_program(KA, KB)

    in_maps = []
    for c in range(NCORES):
        in_maps.append({
            "x_own": x_pad[c],
            "w1cat": w1c, "w2cat": w2c,
            "b1_t": b1_np, "b2m_t": b2m_np,
            "ident_t": ident_np,
            "idxA_in": sch["idxA_w"][c], "idxB_in": sch["idxB_w"][c],
            "oh_in": sch["oh"][c], "ohT_in": sch["ohT"][c],
        })

    res = run_bass_kernel_spmd(nc, in_maps, list(range(NCORES)),
                               trace=bool(os.environ.get("GAT_TRACE")))
    LAST_EXEC_NS = res.exec_time_ns
    out = np.concatenate(
        [res.results[c]["y_out"][:NPC] for c in range(NCORES)], axis=0)
    return out.astype(np.float32)
